# revision 2
# baseline (speedup 1.0000x reference)
"""Causal multi-head attention (B=4, T=2048, C=1024, H=16) on 8 TRN2 cores.

Sharding: batch (4) x head-group (2 groups of 8 heads) -> 8 shards, one per
core. Each core computes QKV projections for its 8 heads, causal flash-style
attention, and a Megatron row-parallel slice of the output projection; the
host sums the two head-group partial outputs per batch element.

v3: all matmuls bf16 (fp32 psum accum). Phase 2 interleaves the two heads of
each head pair so consecutive S matmuls occupy disjoint PE row groups
(rows 0-63 / 64-127, K=64) and execute concurrently. Diagonal staircase is
packed to 1280 columns (sp_a 1024 + half-bank spb 256). Normalization reads
PV psum directly; phase-3 bias rides the idle ACT engine; output is bf16.

Self-contained: hardcodes shapes from the problem spec; no file reads.
"""
import sys
sys.path.insert(0, '/opt/trn_rl_repo')
import numpy as np

B, T, C = 4, 2048, 1024
H, D = 16, 64
N_CORES = 8
HPC = 8        # heads per core
HP = 4         # head pairs per core
KB = 16        # 128-row key tiles per sequence
NQSB = 4       # 512-column query superblocks
CI = 8         # 128-row contraction tiles over C
VW = 66        # V_aug stride per head (64 V + 1 ones + 1 pad)

# Diagonal staircase inside sp_a [128,1024] + spb half-bank [128,256]:
# block j covers query range [QOFF[j], 512), P tile column POFF[j].
QOFF = (0, 128, 256, 384)
POFF = (0, 512, 1024, 896)
PTW = 1280     # packed staircase width (j0 512 + j1 384 + j3 128 + j2 256)

_CACHE = {}


def build_nc(iters=1):
    import contextlib
    import concourse.tile as tile
    from concourse import bacc, mybir

    F32 = mybir.dt.float32
    BF16 = mybir.dt.bfloat16
    EXP = mybir.ActivationFunctionType.Exp
    IDENT = mybir.ActivationFunctionType.Identity

    nc = bacc.Bacc("TRN2", target_bir_lowering=False, debug=False)

    xT_d = nc.dram_tensor("xT", [C, T], BF16, kind="ExternalInput")
    wqT_d = nc.dram_tensor("wqT", [C, 512], BF16, kind="ExternalInput")
    wkT_d = nc.dram_tensor("wkT", [C, 512], BF16, kind="ExternalInput")
    wvT_d = nc.dram_tensor("wvT", [C, 512], BF16, kind="ExternalInput")
    woT_d = nc.dram_tensor("woT", [512, C], BF16, kind="ExternalInput")
    bias_d = nc.dram_tensor("bias", [128, 8], F32, kind="ExternalInput")
    mask_d = nc.dram_tensor("masks", [128, PTW], BF16, kind="ExternalInput")
    yT_d = nc.dram_tensor("yT", [C, T], F32, kind="ExternalOutput")

    with tile.TileContext(nc) as tc:
        def emit():
            with contextlib.ExitStack() as es:
                const = es.enter_context(tc.tile_pool(name="const", bufs=1))
                qtp = es.enter_context(tc.tile_pool(name="qt", bufs=1))
                ctxp = es.enter_context(tc.tile_pool(name="ctx", bufs=1))
                vp = es.enter_context(tc.tile_pool(name="vsb", bufs=1))

                ones_f = const.tile([128, 64], F32)
                nc.any.memset(ones_f[:], 1.0)
                ones_r = const.tile([128, 64], BF16)
                nc.vector.tensor_copy(ones_r[:], ones_f[:])
                ones16_f = const.tile([128, 16], F32)
                nc.any.memset(ones16_f[:], 1.0)
                ones16_r = const.tile([128, 16], BF16)
                nc.vector.tensor_copy(ones16_r[:], ones16_f[:])
                bias_sb = const.tile([128, 8], F32)
                nc.sync.dma_start(bias_sb[:], bias_d.ap())

                ktp = es.enter_context(tc.tile_pool(name="ktp", bufs=1))
                qt_sb, ctx_sb, kt_sb, v_sb = [], [], [], []
                for hp in range(HP):
                    qt_sb.append(qtp.tile([128, T], BF16, tag=f"qt{hp}",
                                          name=f"qt{hp}"))
                    ctx_sb.append(ctxp.tile([128, T], BF16, tag=f"ctx{hp}",
                                            name=f"ctx{hp}"))
                    kt_sb.append(ktp.tile([128, T], BF16, tag=f"kt{hp}",
                                          name=f"kt{hp}"))
                for kb in range(KB):
                    v_sb.append(vp.tile([128, HPC * VW], BF16, tag=f"v{kb}",
                                        name=f"v{kb}"))

                # ---------------- phase 1: projections ----------------
                with contextlib.ExitStack() as p1:
                    xtp = p1.enter_context(tc.tile_pool(name="xt", bufs=1))
                    xt_sb = []
                    for ci in range(CI):
                        t_ = xtp.tile([128, T], BF16, tag=f"xt{ci}")
                        nc.sync.dma_start(t_[:],
                                          xT_d.ap()[ci * 128:(ci + 1) * 128, :])
                        xt_sb.append(t_)

                    # --- 1a: V (resident V_aug tiles) ---
                    with contextlib.ExitStack() as p1a:
                        wvp = p1a.enter_context(tc.tile_pool(name="wv", bufs=1))
                        vps = p1a.enter_context(
                            tc.tile_pool(name="vps", bufs=4, space="PSUM"))
                        wv_sb = []
                        for ci in range(CI):
                            t_ = wvp.tile([128, 512], BF16, tag=f"wv{ci}")
                            nc.sync.dma_start(
                                t_[:], wvT_d.ap()[ci * 128:(ci + 1) * 128, :])
                            wv_sb.append(t_)
                        for ti in range(KB):
                            ps_ = vps.tile([128, 512], F32)
                            for ci in range(CI):
                                nc.tensor.matmul(
                                    ps_[:],
                                    xt_sb[ci][:, ti * 128:(ti + 1) * 128],
                                    wv_sb[ci][:],
                                    start=(ci == 0), stop=(ci == CI - 1),
                                    skip_group_check=True)
                            sv = v_sb[ti][:].rearrange("p (h w) -> p h w", w=VW)
                            nc.vector.tensor_copy(
                                sv[:, :, 64:66],
                                ones16_r[:].rearrange("p (h w) -> p h w", w=2))
                            nc.vector.tensor_copy(
                                sv[:, :, 0:64],
                                ps_[:].rearrange("p (h w) -> p h w", w=64))

                    # --- 1b: Q^T, K^T (per head pair) ---
                    with contextlib.ExitStack() as p1b:
                        wqp = p1b.enter_context(tc.tile_pool(name="wq", bufs=2))
                        wkp = p1b.enter_context(tc.tile_pool(name="wk", bufs=2))
                        qkps = p1b.enter_context(
                            tc.tile_pool(name="qkps", bufs=3, space="PSUM"))
                        for hp in range(HP):
                            fsl = slice(hp * 128, (hp + 1) * 128)
                            wq_sb, wk_sb = [], []
                            for ci in range(CI):
                                tq = wqp.tile([128, 128], BF16, tag=f"wqs{ci}")
                                nc.sync.dma_start(
                                    tq[:], wqT_d.ap()[ci * 128:(ci + 1) * 128, fsl])
                                wq_sb.append(tq)
                                tk = wkp.tile([128, 128], BF16, tag=f"wks{ci}")
                                nc.sync.dma_start(
                                    tk[:], wkT_d.ap()[ci * 128:(ci + 1) * 128, fsl])
                                wk_sb.append(tk)
                            for tj in range(NQSB):
                                tsl = slice(tj * 512, (tj + 1) * 512)
                                ps_ = qkps.tile([128, 512], F32)
                                for ci in range(CI):
                                    nc.tensor.matmul(
                                        ps_[:], wq_sb[ci][:], xt_sb[ci][:, tsl],
                                        start=(ci == 0), stop=(ci == CI - 1),
                                        skip_group_check=True)
                                nc.scalar.copy(qt_sb[hp][:, tsl], ps_[:])
                                ps2 = qkps.tile([128, 512], F32, tag="psk")
                                for ci in range(CI):
                                    nc.tensor.matmul(
                                        ps2[:], wk_sb[ci][:], xt_sb[ci][:, tsl],
                                        start=(ci == 0), stop=(ci == CI - 1),
                                        skip_group_check=True)
                                nc.vector.tensor_copy(
                                    kt_sb[hp][:, tsl], ps2[:])

                # ---------------- phase 2: attention ----------------
                with contextlib.ExitStack() as p2:
                    maskp = p2.enter_context(tc.tile_pool(name="maskp", bufs=1))
                    wop = p2.enter_context(tc.tile_pool(name="wo", bufs=1))
                    ptp = p2.enter_context(tc.tile_pool(name="pt", bufs=4))
                    rrp = p2.enter_context(tc.tile_pool(name="rr", bufs=2))
                    rawp = p2.enter_context(tc.tile_pool(name="raw", bufs=3))
                    tmpp = p2.enter_context(tc.tile_pool(name="tmp", bufs=2))
                    sps0 = p2.enter_context(
                        tc.tile_pool(name="sps0", bufs=1, space="PSUM"))
                    sps1 = p2.enter_context(
                        tc.tile_pool(name="sps1", bufs=1, space="PSUM"))
                    spbp = p2.enter_context(
                        tc.tile_pool(name="spb", bufs=1, space="PSUM"))
                    pvps0 = p2.enter_context(
                        tc.tile_pool(name="pvps0", bufs=1, space="PSUM"))
                    pvps1 = p2.enter_context(
                        tc.tile_pool(name="pvps1", bufs=1, space="PSUM"))
                    bcps = p2.enter_context(
                        tc.tile_pool(name="bcps", bufs=1, space="PSUM"))

                    mask_sb = maskp.tile([128, PTW], BF16)
                    nc.sync.dma_start(mask_sb[:], mask_d.ap())
                    wo_sb = []
                    for hp in range(HP):
                        w_ = wop.tile([128, C], BF16, tag=f"wo{hp}",
                                      name=f"wo{hp}")
                        nc.sync.dma_start(
                            w_[:], woT_d.ap()[hp * 128:(hp + 1) * 128, :])
                        wo_sb.append(w_)

                    PSL = (slice(0, 64), slice(64, 128))
                    for hp in range(HP):
                        kt, qt = kt_sb[hp], qt_sb[hp]
                        for qsb in range(NQSB):
                            qbase = qsb * 512
                            qsl = slice(qbase, qbase + 512)
                            n_full = 4 * qsb
                            vsl = [slice((2 * hp + hl) * VW,
                                         (2 * hp + hl) * VW + 65)
                                   for hl in range(2)]
                            pv = [pvps0.tile([128, 512], F32, tag="pv0",
                                              name="pv0"),
                                  pvps1.tile([128, 512], F32, tag="pv1",
                                             name="pv1")]
                            first = [True, True]
                            for kbp in range(n_full // 2):
                                kb0, kb1 = 2 * kbp, 2 * kbp + 1
                                sp = [sps0.tile([128, 1024], F32, tag="sp0",
                                                name="sp0"),
                                      sps1.tile([128, 1024], F32, tag="sp1",
                                                name="sp1")]
                                for u, kb in enumerate((kb0, kb1)):
                                    for hl in range(2):
                                        nc.tensor.matmul(
                                            sp[hl][:, u * 512:(u + 1) * 512],
                                            kt[PSL[hl], kb * 128:(kb + 1) * 128],
                                            qt[PSL[hl], qsl],
                                            start=True, stop=True,
                                            skip_group_check=True)
                                pt = [None, None]
                                for hl in range(2):
                                    pt[hl] = ptp.tile([128, PTW], BF16,
                                                      tag="pt", name="pt")
                                    nc.scalar.activation(
                                        pt[hl][:, 0:1024], sp[hl][:],
                                        EXP, scale=0.125)
                                for u, kb in enumerate((kb0, kb1)):
                                    for hl in range(2):
                                        nc.tensor.matmul(
                                            pv[hl][0:65, :],
                                            v_sb[kb][:, vsl[hl]],
                                            pt[hl][:, u * 512:(u + 1) * 512],
                                            start=first[hl], stop=False,
                                            skip_group_check=True)
                                        first[hl] = False
                            # diagonal staircase: j0,j1,j3 in sp_a [128,1024],
                            # j2 in a half-bank slice of spb (h0 lo, h1 hi)
                            sp_a = [sps0.tile([128, 1024], F32, tag="sp0",
                                              name="spa0"),
                                    sps1.tile([128, 1024], F32, tag="sp1",
                                              name="spa1")]
                            for j in (0, 1, 3):
                                kb = n_full + j
                                n_ = 512 - QOFF[j]
                                for hl in range(2):
                                    nc.tensor.matmul(
                                        sp_a[hl][:, POFF[j]:POFF[j] + n_],
                                        kt[PSL[hl], kb * 128:(kb + 1) * 128],
                                        qt[PSL[hl],
                                           qbase + QOFF[j]:qbase + 512],
                                        start=True, stop=True,
                                        skip_group_check=True)
                            pt = [None, None]
                            for hl in range(2):
                                # per-head spb from a bufs=1 pool: the second
                                # head's write is serialized behind the first
                                # head's exp read (same-bank WAR), so two
                                # row-group-concurrent matmuls never drain
                                # into this bank at the same time
                                spb = spbp.tile([128, 256], F32, tag="spb",
                                                name="spb")
                                kb2 = n_full + 2
                                nc.tensor.matmul(
                                    spb[:],
                                    kt[PSL[hl], kb2 * 128:(kb2 + 1) * 128],
                                    qt[PSL[hl],
                                       qbase + QOFF[2]:qbase + 512],
                                    start=True, stop=True,
                                    skip_group_check=True)
                                p_ = ptp.tile([128, PTW], BF16, tag="pt")
                                nc.scalar.activation(p_[:, 0:1024],
                                                     sp_a[hl][:],
                                                     EXP, scale=0.125)
                                nc.scalar.activation(p_[:, 1024:PTW],
                                                     spb[:],
                                                     EXP, scale=0.125)
                                nc.vector.tensor_mul(p_[:], p_[:], mask_sb[:])
                                pt[hl] = p_
                                for j in (0, 1, 3, 2):
                                    kb = n_full + j
                                    n_ = 512 - QOFF[j]
                                    nc.tensor.matmul(
                                        pv[hl][0:65, QOFF[j]:512],
                                        v_sb[kb][:, vsl[hl]],
                                        pt[hl][:, POFF[j]:POFF[j] + n_],
                                        start=first[hl], stop=(j == 2),
                                        skip_group_check=True)
                                    first[hl] = False
                            # normalize: ctx = pv[0:64] * (1 / pv[64]),
                            # denominators broadcast across partitions via PE
                            for hl in range(2):
                                raw = rawp.tile([65, 512], F32, tag="raw")
                                nc.vector.tensor_copy(raw[0:64, :],
                                                      pv[hl][0:64, :])
                                rr = rrp.tile([65, 512], BF16, tag="rr")
                                with nc.allow_low_precision("softmax denom"):
                                    nc.vector.reciprocal(rr[64:65, :],
                                                         pv[hl][64:65, :])
                                bc = bcps.tile([64, 512], F32, tag="bc")
                                nc.tensor.matmul(bc[:], ones_r[64:65, :],
                                                 rr[64:65, :],
                                                 start=True, stop=True,
                                                 skip_group_check=True)
                                if hl == 0:
                                    nc.vector.tensor_mul(
                                        ctx_sb[hp][0:64, qsl],
                                        raw[0:64, :], bc[:])
                                else:
                                    tmp = tmpp.tile([64, 512], BF16)
                                    nc.vector.tensor_mul(tmp[:],
                                                         raw[0:64, :],
                                                         bc[:])
                                    nc.sync.dma_start(
                                        ctx_sb[hp][64:128, qsl], tmp[:])

                    # -------------- phase 3: output projection --------------
                    with contextlib.ExitStack() as p3:
                        yp = p3.enter_context(tc.tile_pool(name="y", bufs=3))
                        for oi in range(8):
                            osl = slice(oi * 128, (oi + 1) * 128)
                            for tj in range(NQSB):
                                tsl = slice(tj * 512, (tj + 1) * 512)
                                yps = pvps0 if (oi * NQSB + tj) % 2 == 0 \
                                    else pvps1
                                ps_ = yps.tile([128, 512], F32,
                                               tag="pv0" if yps is pvps0
                                               else "pv1", name="yacc")
                                for hp in range(HP):
                                    nc.tensor.matmul(
                                        ps_[:], wo_sb[hp][:, osl],
                                        ctx_sb[hp][:, tsl],
                                        start=(hp == 0), stop=(hp == HP - 1),
                                        skip_group_check=True)
                                y_ = yp.tile([128, 512], F32)
                                nc.scalar.activation(
                                    y_[:], ps_[:], IDENT,
                                    bias=bias_sb[:, oi:oi + 1])
                                nc.sync.dma_start(yT_d.ap()[osl, tsl], y_[:])

        if iters == 1:
            emit()
        else:
            with tc.For_i(0, iters, 1):
                emit()
    nc.compile()
    return nc


def make_masks():
    """Packed staircase mask [128, PTW]: pt col POFF[j] + (q - QOFF[j])
    holds causal keep-bit for key row k = 128*j + k_local vs query q."""
    m = np.zeros((128, PTW), np.float32)
    k = np.arange(128)[:, None]
    for j in range(4):
        q = np.arange(QOFF[j], 512)[None, :]
        m[:, POFF[j]:POFF[j] + 512 - QOFF[j]] = (q >= 128 * j + k)
    return m


def shard_inputs(x, w_qkv, w_out, b_out):
    """Full inputs -> list of 8 per-core input dicts."""
    import ml_dtypes
    bf16 = ml_dtypes.bfloat16
    x = np.asarray(x, dtype=np.float32).astype(bf16)
    w_qkv = np.asarray(w_qkv, dtype=np.float32).astype(bf16)
    w_out = np.asarray(w_out, dtype=np.float32).astype(bf16)
    b_out = np.asarray(b_out, dtype=np.float32)
    masks = make_masks().astype(bf16)
    in_maps = []
    for c in range(N_CORES):
        b, hg = c // 2, c % 2
        h0 = hg * HPC
        csl = slice(h0 * D, (h0 + HPC) * D)
        im = {
            "xT": np.ascontiguousarray(x[b].T),
            "wqT": np.ascontiguousarray(w_qkv[0 * C:1 * C][csl].T),
            "wkT": np.ascontiguousarray(w_qkv[1 * C:2 * C][csl].T),
            "wvT": np.ascontiguousarray(w_qkv[2 * C:3 * C][csl].T),
            "woT": np.ascontiguousarray(w_out[:, csl].T),
            "bias": (np.ascontiguousarray(
                b_out.reshape(8, 128).T.astype(np.float32))
                     if hg == 0 else np.zeros((128, 8), np.float32)),
            "masks": masks,
        }
        in_maps.append(im)
    return in_maps


def gather_outputs(results):
    """8 per-core {'yT': [C,T]} -> full [B,T,C]."""
    y = np.empty((B, T, C), np.float32)
    for b in range(B):
        acc = (results[2 * b]["yT"].astype(np.float32)
               + results[2 * b + 1]["yT"].astype(np.float32))
        y[b] = acc.T
    return y


def kernel(**inputs):
    from concourse.bass_utils import run_bass_kernel_spmd
    if "nc" not in _CACHE:
        _CACHE["nc"] = build_nc()
    nc = _CACHE["nc"]
    in_maps = shard_inputs(inputs["x"], inputs["w_qkv"],
                           inputs["w_out"], inputs["b_out"])
    res = run_bass_kernel_spmd(nc, in_maps, list(range(N_CORES)))
    return gather_outputs(res.results)
